# revision 1
# baseline (speedup 1.0000x reference)
"""Trainium2 Bass kernel for nn_Net_67954972557347 (dense_mlp).

Network: a1 = lrelu(a@Wa+ba) [B,68]; b1 = lrelu(b@Wb+bb) [B,68];
c = [a1|b1|meta] [B,140]; then 10 lrelu'd dense layers
(140->34->34->20->20->20->20->20->5->2->1), lrelu slope 0.01.

Strategy: pure data parallel over 8 cores (32768 rows each). On-device,
activations are feature-major ([feat, batch]) so each layer is a PE matmul
with the batch streaming as the moving operand (float32r / tf32 datapath,
fp32 PSUM accumulation). The host pre-transposes and packs the inputs:

  t1 [128, 32768]: rows 0:45 = a.T, 45:49 = ilrelu(meta.T),
                   rows 49:128 = b.T[0:79]
  t2 [128, 8192]:  per chunk c (512 cols): 32-row block c%4 of column
                   group c//4 = [b.T[79:102]; ones] (24 rows)

Every matmul reads a base-0 partition window of one SBUF tile with
zero-padded weight columns/rows (this walrus build only supports
tile_position (0,0)-style full windows reliably), and every matmul output
starts at PSUM partition 0. meta rides through the first-layer matmul as a
passthrough output (host pre-applies inverse-lrelu so the drain's
leaky-relu recovers it exactly); the ones row in t2 folds the b1 bias into
the B matmul.

A 10-step software pipeline processes one 512-column chunk per step with
4 PSUM banks (chunk ages at step t):
  alpha [0:72]  = [a1(68); meta(4)]              of chunk t     (1 MM)
  beta  [0:68]  = b1                             of chunk t     (2 MMs)
  dA    [0:68]  = [c0(t-1); c1(t-2)]                            (3 MMs)
  dB    [0:108] = [y(t-10); c2(t-3); c3(t-4); c4(t-5); c5(t-6);
                   c6(t-7); c7(t-8); c8(t-9)]                   (2 MMs)
The whole deep tail (c2->...->c8->y) advances one stage per step inside a
single matmul (D2) whose block-structured weights read the previous step's
dB drain. alpha/dA/dB are drained by one ACT Prelu each (per-partition
bias vector, alpha=0.01); beta drains on DVE (copy + max(0.01x, x), bias
pre-folded via the ones row).
"""

import os
import sys

import numpy as np

for _p in ("/opt/trn_rl_repo", "/root/.axon_site/_ro/trn_rl_repo"):
    if os.path.isdir(_p) and _p not in sys.path:
        sys.path.append(_p)

import concourse.bass as bass
import concourse.mybir as mybir
import concourse.tile as tile
from concourse import bacc
from concourse.bass_utils import run_bass_kernel_spmd
from bass_rust import add_dep_helper

F32 = mybir.dt.float32
F32R = mybir.dt.float32r
ALU = mybir.AluOpType
PRELU = mybir.ActivationFunctionType.Prelu

B_FULL = 262144
N_CORES = 8
B_CORE = B_FULL // N_CORES          # 32768
N = 512                              # columns per chunk (PSUM bank / fp32 cap)
PIPE = 10                            # pipeline depth in steps
ALPHA = 0.01                         # leaky-relu slope

# weight-tile column spans (B2 has 4 spans of 68 at CB2 + 68*k)
CA1, CB1, CB2 = 0, 72, 140
CL0A, CL0B, CL1 = 412, 480, 514
CD1, CD2 = 582, 690
WT_COLS = 1024
M_AL, M_BE, M_DA, M_DB = 72, 68, 68, 108


def _ilrelu(x):
    """Inverse of leaky-relu (slope 0.01)."""
    return np.where(x > 0, x, x * (1.0 / ALPHA)).astype(np.float32)


def _pack_weights(Wa, ba, Wb, bb, Ws, Bs):
    """Build the [128, WT_COLS] packed weight tile and [128, 3] bias tile."""
    W0, W1, W2, W3, W4, W5, W6, W7, W8, W9 = Ws
    B0, B1, B2, B3, B4, B5, B6, B7, B8, B9 = Bs
    wt = np.zeros((128, WT_COLS), np.float32)
    # A1: rhs t1[0:49]: rows 0:45 = a.T -> a1 (cols 0:68);
    # rows 45:49 = meta passthrough (cols 68:72)
    wt[0:45, CA1:CA1 + 68] = Wa
    wt[45:49, CA1 + 68:CA1 + 72] = np.eye(4, dtype=np.float32)
    # B1: rhs t1[0:128]: rows 49:128 = b.T[0:79]
    wt[49:128, CB1:CB1 + 68] = Wb[0:79]
    # B2 (span per k): rhs t2[0:128]: block k rows 32k:32k+23 = b.T[79:102],
    # row 32k+23 = ones -> bias bb
    for kk in range(4):
        r = 32 * kk
        c = CB2 + 68 * kk
        wt[r:r + 23, c:c + 68] = Wb[79:102]
        wt[r + 23, c:c + 68] = bb
    # L0a: rhs a1t[0:72]: a1 -> W0[0:68], meta -> W0[136:140]; c0 cols 0:34
    wt[0:68, CL0A:CL0A + 34] = W0[0:68]
    wt[68:72, CL0A:CL0A + 34] = W0[136:140]
    # L0b: rhs b1t[0:68] -> c0 (cols 0:34)
    wt[0:68, CL0B:CL0B + 34] = W0[68:136]
    # L1: rhs DA[0:34] = c0 -> c1 (cols 34:68)
    wt[0:34, CL1 + 34:CL1 + 68] = W1
    # D1: rhs DA[0:68]: rows 34:68 = c1 -> c2 (cols 1:21)
    wt[34:68, CD1 + 1:CD1 + 21] = W2
    # D2: rhs DB[0:108]: the whole tail chain advances one stage
    wt[1:21, CD2 + 21:CD2 + 41] = W3      # c2 -> c3
    wt[21:41, CD2 + 41:CD2 + 61] = W4     # c3 -> c4
    wt[41:61, CD2 + 61:CD2 + 81] = W5     # c4 -> c5
    wt[61:81, CD2 + 81:CD2 + 101] = W6    # c5 -> c6
    wt[81:101, CD2 + 101:CD2 + 106] = W7  # c6 -> c7
    wt[101:106, CD2 + 106:CD2 + 108] = W8  # c7 -> c8
    wt[106:108, CD2:CD2 + 1] = W9         # c8 -> y

    bias = np.zeros((128, 3), np.float32)
    bias[0:68, 0] = ba                    # alpha bank
    bias[0:34, 1] = B0                    # dA bank
    bias[34:68, 1] = B1
    bias[0:1, 2] = B9                     # dB bank
    bias[1:21, 2] = B2
    bias[21:41, 2] = B3
    bias[41:61, 2] = B4
    bias[61:81, 2] = B5
    bias[81:101, 2] = B6
    bias[101:106, 2] = B7
    bias[106:108, 2] = B8
    return wt, bias


def _pack_core_inputs(a, b, meta, n_chunks):
    """Pack one core's shard into the t1/t2 DMA streams."""
    bc = n_chunks * N
    t1 = np.empty((128, bc), np.float32)
    t1[0:45] = a[:bc].T
    t1[45:49] = _ilrelu(meta[:bc].T)
    t1[49:128] = b[:bc, 0:79].T
    n_super = (n_chunks + 3) // 4
    t2 = np.zeros((128, n_super * N), np.float32)
    bT_tail = np.ascontiguousarray(b[:bc, 79:102].T)
    for c in range(n_chunks):
        r = 32 * (c % 4)
        cs = slice(c * N, (c + 1) * N)
        ds = slice((c // 4) * N, (c // 4 + 1) * N)
        t2[r:r + 23, ds] = bT_tail[:, cs]
        t2[r + 23, ds] = 1.0
    return t1, t2


def build_bass(n_chunks):
    """Build + compile the per-core Bass program (same on all 8 cores)."""
    nc = bacc.Bacc(None, target_bir_lowering=False, debug=False)
    n_steps = n_chunks + PIPE
    n_super = (n_chunks + 3) // 4

    t1_d = nc.dram_tensor("t1", [128, n_chunks * N], F32,
                          kind="ExternalInput")
    t2_d = nc.dram_tensor("t2", [128, n_super * N], F32,
                          kind="ExternalInput")
    wt_d = nc.dram_tensor("wt", [128, WT_COLS], F32, kind="ExternalInput")
    bias_d = nc.dram_tensor("bias", [128, 3], F32, kind="ExternalInput")
    y_d = nc.dram_tensor("y", [1, n_chunks * N], F32, kind="ExternalOutput")

    with tile.TileContext(nc) as tc:
        with (
            tc.tile_pool(name="const", bufs=1) as constp,
            tc.tile_pool(name="t1p", bufs=3) as t1p,
            tc.tile_pool(name="t2p", bufs=2) as t2p,
            tc.tile_pool(name="actp", bufs=3) as actp,
            tc.tile_pool(name="dp", bufs=2) as dp,
            tc.tile_pool(name="ps", bufs=2, space=bass.MemorySpace.PSUM) as ps,
        ):
            wt = constp.tile([128, WT_COLS], F32R, tag="wt")
            bias = constp.tile([128, 3], F32, tag="bias")
            z1 = constp.tile([128, N], F32R, tag="z1")
            nc.sync.dma_start(wt[:], wt_d[:].bitcast(F32R))
            nc.sync.dma_start(bias[:], bias_d[:])
            nc.gpsimd.memset(z1[:].bitcast(F32), 0.0)

            def w(c0, c1):
                return wt[:, c0:c1]

            def chain(*insts):
                for i in range(1, len(insts)):
                    add_dep_helper(insts[i].ins, insts[i - 1].ins,
                                   sync=False, reason="psum acc order")

            t1s, t2s, a1s, b1s, das, dbs = {}, {}, {}, {}, {}, {}
            for d, pool, tag in ((a1s, actp, "a1"), (b1s, actp, "b1"),
                                 (das, dp, "da"), (dbs, dp, "db")):
                d[-1] = pool.tile([128, N], F32R, tag=tag, name=f"{tag}_zm1")
                nc.gpsimd.memset(d[-1][:].bitcast(F32), 0.0)

            for t in range(n_steps):
                # ---- DMAs in ----
                if t < n_chunks:
                    t1s[t] = t1p.tile([128, N], F32R, tag="t1",
                                      name=f"t1_{t}")
                    nc.sync.dma_start(
                        t1s[t][:], t1_d[:, t * N:(t + 1) * N].bitcast(F32R))
                    if t % 4 == 0:
                        s = t // 4
                        t2s[s] = t2p.tile([128, N], F32R, tag="t2",
                                          name=f"t2_{s}")
                        nc.sync.dma_start(
                            t2s[s][:],
                            t2_d[:, s * N:(s + 1) * N].bitcast(F32R))

                mm = nc.tensor.matmul
                k = t % 4
                rhs1 = t1s[t][:] if t < n_chunks else z1[:]

                al = ps.tile([128, N], F32, tag="al", name=f"al_{t}")
                be = ps.tile([128, N], F32, tag="be", name=f"be_{t}")
                dA = ps.tile([128, N], F32, tag="dA", name=f"dA_{t}")
                dB = ps.tile([128, N], F32, tag="dB", name=f"dB_{t}")

                # ---- alpha: A1 (a1 + meta passthrough) ----
                mm(al[0:M_AL], w(CA1, CA1 + M_AL)[0:49], rhs1[0:49],
                   start=True, stop=True, tile_position=(0, 0))

                # ---- beta: B1 + B2 (b-tail + ones->bias) ----
                i1 = mm(be[0:M_BE], w(CB1, CB1 + M_BE)[0:128], rhs1,
                        start=True, stop=t >= n_chunks,
                        tile_position=(0, 0))
                if t < n_chunks:
                    cb2 = CB2 + 68 * k
                    i2 = mm(be[0:M_BE], wt[0:128, cb2:cb2 + M_BE],
                            t2s[t // 4][0:128],
                            start=False, stop=True, tile_position=(0, 0))
                    chain(i1, i2)

                # ---- dA: L0a + L0b (c0), L1 (c1) ----
                i1 = mm(dA[0:M_DA], w(CL0A, CL0A + M_DA)[0:M_AL],
                        a1s[t - 1][0:M_AL],
                        start=True, stop=False, tile_position=(0, 0))
                i2 = mm(dA[0:34], w(CL0B, CL0B + 34)[0:M_BE],
                        b1s[t - 1][0:M_BE],
                        start=False, stop=False, tile_position=(0, 0))
                i3 = mm(dA[0:M_DA], w(CL1, CL1 + M_DA)[0:34],
                        das[t - 1][0:34],
                        start=False, stop=True, tile_position=(0, 0))
                chain(i1, i2, i3)

                # ---- dB: D1 (c2), D2 (tail chain c3..c8, y) ----
                i1 = mm(dB[0:M_DB], w(CD1, CD1 + M_DB)[0:M_DA],
                        das[t - 1][0:M_DA],
                        start=True, stop=False, tile_position=(0, 0))
                i2 = mm(dB[0:M_DB], w(CD2, CD2 + M_DB)[0:M_DB],
                        dbs[t - 1][0:M_DB],
                        start=False, stop=True, tile_position=(0, 0))
                chain(i1, i2)

                # ---- drains ----
                a1s[t] = actp.tile([128, N], F32R, tag="a1", name=f"a1_{t}")
                nc.scalar.activation(a1s[t][0:M_AL], al[0:M_AL], PRELU,
                                     bias=bias[0:M_AL, 0:1], alpha=ALPHA)
                das[t] = dp.tile([128, N], F32R, tag="da", name=f"da_{t}")
                nc.scalar.activation(das[t][0:M_DA], dA[0:M_DA], PRELU,
                                     bias=bias[0:M_DA, 1:2], alpha=ALPHA)
                dbs[t] = dp.tile([128, N], F32R, tag="db", name=f"db_{t}")
                nc.scalar.activation(dbs[t][0:M_DB], dB[0:M_DB], PRELU,
                                     bias=bias[0:M_DB, 2:3], alpha=ALPHA)
                b1s[t] = actp.tile([128, N], F32R, tag="b1", name=f"b1_{t}")
                nc.vector.tensor_copy(b1s[t][0:M_BE], be[0:M_BE])
                nc.vector.scalar_tensor_tensor(
                    b1s[t][0:M_BE], b1s[t][0:M_BE], ALPHA, b1s[t][0:M_BE],
                    ALU.mult, ALU.max)

                # ---- y out ----
                if t >= PIPE:
                    c = t - PIPE
                    nc.gpsimd.dma_start(
                        y_d[:, c * N:(c + 1) * N].bitcast(F32R),
                        dbs[t][0:1])

    nc.compile()
    return nc


_NC_CACHE = {}


def _get_nc(n_chunks):
    if n_chunks not in _NC_CACHE:
        _NC_CACHE[n_chunks] = build_bass(n_chunks)
    return _NC_CACHE[n_chunks]


def run_cores(inputs, n_chunks, cores, trace=False, trace_kwargs=None):
    """Pack inputs, run the SPMD kernel on the given cores, return
    (per-core y arrays, BassKernelResults)."""
    a = np.asarray(inputs["a"], np.float32)
    b = np.asarray(inputs["b"], np.float32)
    meta = np.asarray(inputs["meta"], np.float32)
    Ws = [np.asarray(inputs[f"W{i}"], np.float32) for i in range(10)]
    Bs = [np.asarray(inputs[f"B{i}"], np.float32) for i in range(10)]
    wt, bias = _pack_weights(np.asarray(inputs["Wa"], np.float32),
                             np.asarray(inputs["ba"], np.float32),
                             np.asarray(inputs["Wb"], np.float32),
                             np.asarray(inputs["bb"], np.float32), Ws, Bs)
    in_maps = []
    for r in cores:
        sl = slice(r * B_CORE, r * B_CORE + n_chunks * N)
        t1, t2 = _pack_core_inputs(a[sl], b[sl], meta[sl], n_chunks)
        in_maps.append({"t1": t1, "t2": t2, "wt": wt, "bias": bias})
    nc = _get_nc(n_chunks)
    kw = dict(trace=trace)
    if trace_kwargs:
        kw.update(trace_kwargs)
    res = run_bass_kernel_spmd(nc, in_maps, list(range(len(cores))), **kw)
    return [res.results[i]["y"] for i in range(len(cores))], res


def kernel(**inputs):
    n_chunks = B_CORE // N
    ys, _ = run_cores(inputs, n_chunks, list(range(N_CORES)))
    out = np.empty((B_FULL, 1), np.float32)
    for r in range(N_CORES):
        out[r * B_CORE:(r + 1) * B_CORE, 0] = ys[r][0]
    return out

